# revision 20
# baseline (speedup 1.0000x reference)
"""Bahdanau additive attention on 8 Trainium2 NeuronCores (data-parallel).

Reference computation (per batch item b):
    t[c]      = target[b] @ W1_w + W1_b                         # [C]
    s[n, c]   = sources[b] @ W2_w + W2_b                        # [N, C]
    score[n]  = (tanh(t + s)[n, :] @ V_w + V_b) / sqrt(C)       # [N]
    attw      = softmax(score)                                  # [N]
    ctx[h]    = sum_n attw[n] * sources[b, n, h]                # [H]

Strategy: shard the batch dim (1024) over 8 cores (128 items each),
replicate the small weights.  Per item everything stays on-chip:
  - sources loaded once via SWDGE cast-DMA (f32 -> bf16), natural [n, h]
    layout; also reused as the ctx-matmul rhs.
  - the contraction-over-h matmul needs sources transposed: done on the
    TensorEngine as a regular matmul against the identity (out =
    natT @ I), 128x128 blocks, evacuated PSUM -> SBUF bf16 by the DVE.
    (DMA-transpose would be cheaper on paper, but this Tile snapshot
    does not serialize the xbar transpose mode against plain DMA copies
    - a known hardware hang - so it is avoided entirely.)
  - s^T[c, n] accumulates in PSUM; ScalarE applies tanh with the
    per-partition bias t^T[c] + W1_b + W2_b fused in (and evacuates
    PSUM -> SBUF bf16).
  - score^T[n] via 16 small matmuls (lhsT = tanh tiles, rhs = V chunks).
  - exp on ScalarE with the 1/sqrt(C) scale and V_b bias folded in.
  - softmax skips the max-subtraction (|score| <= sum|V|/sqrt(C) < ~1,
    exp is safe and the result is mathematically identical).
  - ctx_u and Z = sum(e) via matmuls against the natural-layout sources;
    normalization on-chip with DVE reciprocal.
attw is staged on-chip in [p, c, b] layout (n = c*128 + p) and fixed up
with a host-side transpose after gathering.
"""

import math
import numpy as np

B, N, H, C = 1024, 512, 512, 512
NCORES = 8
P = 128
NCH, HCH, CCH = N // P, H // P, C // P
B_SHARD = B // NCORES

_build_cache = {}


def build_nc(b_shard=B_SHARD, enable_asserts=False):
    """Build (and bacc-compile) the per-core Bass graph."""
    key = (b_shard, enable_asserts)
    if key in _build_cache:
        return _build_cache[key]

    from contextlib import ExitStack

    import concourse.bass as bass
    import concourse.tile as tile
    from concourse import bacc, mybir

    FP32 = mybir.dt.float32
    BF16 = mybir.dt.bfloat16
    AF = mybir.ActivationFunctionType
    RSQRT_C = 1.0 / math.sqrt(C)

    nc = bacc.Bacc(
        "TRN2", target_bir_lowering=False, debug=False, enable_asserts=enable_asserts
    )

    tgt = nc.declare_dram_parameter("target", [b_shard, H], FP32, isOutput=False)
    src = nc.declare_dram_parameter("sources", [b_shard, N, H], FP32, isOutput=False)
    w1 = nc.declare_dram_parameter("W1_w", [H, C], FP32, isOutput=False)
    w1b = nc.declare_dram_parameter("W1_b", [C], FP32, isOutput=False)
    w2 = nc.declare_dram_parameter("W2_w", [H, C], FP32, isOutput=False)
    w2b = nc.declare_dram_parameter("W2_b", [C], FP32, isOutput=False)
    vw = nc.declare_dram_parameter("V_w", [C, 1], FP32, isOutput=False)
    vb = nc.declare_dram_parameter("V_b", [1], FP32, isOutput=False)
    out_ctx = nc.declare_dram_parameter("ctx", [b_shard, H], FP32, isOutput=True)
    out_attw = nc.declare_dram_parameter(
        "attw", [P, NCH, b_shard], FP32, isOutput=True
    )

    with tile.TileContext(nc) as tc, ExitStack() as ctx:
        singles = ctx.enter_context(tc.tile_pool(name="singles", bufs=1))
        natp = ctx.enter_context(tc.tile_pool(name="nat", bufs=4))
        srcTp = ctx.enter_context(tc.tile_pool(name="srcT", bufs=10))
        tanp = ctx.enter_context(tc.tile_pool(name="tanh", bufs=8))
        ep = ctx.enter_context(tc.tile_pool(name="e", bufs=6))
        ctxsbp = ctx.enter_context(tc.tile_pool(name="ctxsb", bufs=6))
        smallp = ctx.enter_context(tc.tile_pool(name="small", bufs=8))
        # PSUM pools: 4 + 2 + 1 + 1 = 8 banks of 8
        psp = ctx.enter_context(tc.tile_pool(name="ps_s", bufs=4, space="PSUM"))
        psTp = ctx.enter_context(tc.tile_pool(name="ps_T", bufs=2, space="PSUM"))
        scp = ctx.enter_context(tc.tile_pool(name="ps_sc", bufs=1, space="PSUM"))
        ctxp = ctx.enter_context(tc.tile_pool(name="ps_ctx", bufs=1, space="PSUM"))

        # ---- one-time per-core constants ----
        # W2 as bf16 lhsT blocks: w2_sb[p, k, i, c'] = W2[128k+p, 128i+c']
        w2_sb = singles.tile([P, HCH, CCH, P], BF16)
        nc.gpsimd.dma_start(
            out=w2_sb, in_=w2[:, :].rearrange("(k p) (i c) -> p k i c", p=P, c=P)
        )
        w1_sb = singles.tile([P, HCH, CCH, P], BF16)
        nc.gpsimd.dma_start(
            out=w1_sb, in_=w1[:, :].rearrange("(k p) (i c) -> p k i c", p=P, c=P)
        )
        # biases chunked [q, i] = b[128i + q]
        w1b_sb = singles.tile([P, CCH], FP32)
        nc.gpsimd.dma_start(out=w1b_sb, in_=w1b[:].rearrange("(i q) -> q i", q=P))
        w2b_sb = singles.tile([P, CCH], FP32)
        nc.gpsimd.dma_start(out=w2b_sb, in_=w2b[:].rearrange("(i q) -> q i", q=P))
        bsum = singles.tile([P, CCH], FP32)
        nc.vector.tensor_add(bsum, w1b_sb, w2b_sb)
        # V chunked [q, i] = V_w[128i + q]
        v_sb = singles.tile([P, CCH], BF16)
        nc.gpsimd.dma_start(out=v_sb, in_=vw[:, :].rearrange("(i q) o -> q (i o)", q=P))
        # V_b broadcast to all partitions, pre-scaled by 1/sqrt(C)
        vbs = singles.tile([P, 1], FP32)
        vb_ap = bass.AP(tensor=vb[:].tensor, offset=0, ap=[[0, P], [1, 1]])
        nc.gpsimd.dma_start(out=vbs, in_=vb_ap)
        nc.vector.tensor_scalar_mul(vbs, vbs, RSQRT_C)
        # constants for the Z-sum and the reciprocal broadcast
        ones_col = singles.tile([P, 1], BF16)
        nc.vector.memset(ones_col, 1.0)
        ones_row = singles.tile([1, P], FP32)
        nc.vector.memset(ones_row, 1.0)
        # per-item 1/Z collected here; attw is normalized once at the end
        rz_row = singles.tile([1, b_shard], FP32)
        # bf16 identity for TensorEngine transposes
        from concourse import masks

        idn = singles.tile([P, P], BF16)
        masks.make_identity(nc, idn[:])
        # attw staging: [p, c, b] = unnormalized exp(score), n = 128c+p.
        # bf16 so the ctx matmul can consume slices directly as lhsT.
        attw_stage = singles.tile([P, NCH, b_shard], BF16)
        attw_norm = singles.tile([P, NCH, b_shard], FP32)

        # ---- t^T = (target @ W1 + W1_b + W2_b)^T, [q, i, b] layout ----
        tgt_bf = singles.tile([b_shard, H], BF16)
        nc.gpsimd.dma_start(out=tgt_bf, in_=tgt[:, :])
        tgtT = singles.tile([P, HCH, b_shard], BF16)
        for k in range(HCH):
            ptT = psTp.tile([P, b_shard], FP32, tag="psT")
            nc.tensor.matmul(
                ptT,
                lhsT=tgt_bf[:, k * P : (k + 1) * P],
                rhs=idn[:b_shard, :b_shard],
                start=True,
                stop=True,
            )
            nc.vector.tensor_copy(tgtT[:, k, :], ptT)
        tT_sb = singles.tile([P, CCH, b_shard], FP32)
        for i in range(CCH):
            pt = psp.tile([P, b_shard], FP32, tag="ps")
            for k in range(HCH):
                nc.tensor.matmul(
                    pt,
                    lhsT=w1_sb[:, k, i, :],
                    rhs=tgtT[:, k, :],
                    start=(k == 0),
                    stop=(k == HCH - 1),
                )
            nc.scalar.activation(
                tT_sb[:, i, :], pt, AF.Identity, bias=bsum[:, i : i + 1]
            )

        # ---- per batch item ----
        for b in range(b_shard):
            # sources[b] in natural layout, bf16: nat[p, c, h] = src[b, 128c+p, h]
            nat = natp.tile([P, NCH, H], BF16)
            nc.gpsimd.dma_start(
                out=nat, in_=src[b].rearrange("(c p) h -> p c h", p=P)
            )
            # transposed: srcT_j[q, n] = src[b, n, 128j+q]
            # (regular matmul against identity: out = nat_block^T @ I)
            srcTs = []
            for j in range(HCH):
                psT = psTp.tile([P, N], FP32, tag="psT")
                for c in range(NCH):
                    nc.tensor.matmul(
                        psT[:, c * P : (c + 1) * P],
                        lhsT=nat[:, c, j * P : (j + 1) * P],
                        rhs=idn,
                        start=True,
                        stop=True,
                    )
                sT = srcTp.tile([P, N], BF16)
                nc.vector.tensor_copy(sT, psT)
                srcTs.append(sT)

            # score^T accumulator + Z + rz broadcast share one PSUM bank
            sc = scp.tile([P, NCH + 2], FP32)
            ths = []
            for i in range(CCH):
                ps = psp.tile([P, N], FP32, tag="ps")
                for j in range(HCH):
                    nc.tensor.matmul(
                        ps,
                        lhsT=w2_sb[:, j, i, :],
                        rhs=srcTs[j],
                        start=(j == 0),
                        stop=(j == HCH - 1),
                    )
                th = tanp.tile([P, N], BF16)
                nc.scalar.activation(th, ps, AF.Tanh, bias=tT_sb[:, i, b : b + 1])
                ths.append(th)
            # score column c must finish its accumulation group before the
            # next column starts (one pending group per PSUM bank).
            for c in range(NCH):
                for i in range(CCH):
                    nc.tensor.matmul(
                        sc[:, c : c + 1],
                        lhsT=ths[i][:, c * P : (c + 1) * P],
                        rhs=v_sb[:, i : i + 1],
                        start=(i == 0),
                        stop=(i == CCH - 1),
                    )

            # e = exp(score / sqrt(C) + V_b / sqrt(C)) written straight into
            # the (unnormalized) attw staging buffer; per-partition sums ride
            # along via accum_out, so Z needs only one 128->1 matmul.
            esum = ep.tile([P, 1], FP32)
            nc.scalar.activation(
                attw_stage[:, :, b],
                sc[:, 0:NCH],
                AF.Exp,
                bias=vbs[:, 0:1],
                scale=RSQRT_C,
                accum_out=esum,
            )
            esum16 = ep.tile([P, 1], BF16)
            nc.vector.tensor_copy(esum16, esum)

            # ctx_u[0, h] = sum_n e[n] src[b, n, h];  Z = sum_n e[n]
            pctx = ctxp.tile([1, H], FP32)
            for c in range(NCH):
                nc.tensor.matmul(
                    pctx,
                    lhsT=attw_stage[:, c, b : b + 1],
                    rhs=nat[:, c, :],
                    start=(c == 0),
                    stop=(c == NCH - 1),
                )
            nc.tensor.matmul(
                sc[0:1, NCH : NCH + 1],
                lhsT=esum16,
                rhs=ones_col,
                start=True,
                stop=True,
            )
            rz = smallp.tile([1, 1], FP32)
            nc.vector.reciprocal(rz, sc[0:1, NCH : NCH + 1])
            nc.vector.tensor_copy(rz_row[0:1, b : b + 1], rz)
            ctx_sb = ctxsbp.tile([1, H], FP32)
            nc.vector.tensor_scalar_mul(ctx_sb, pctx, rz)
            nc.sync.dma_start(out=out_ctx[b : b + 1, :], in_=ctx_sb)

        # normalize attw once: attw_norm[p, c, b] = stage[p, c, b] * rz[b]
        prz = psTp.tile([P, b_shard], FP32, tag="psT")
        nc.tensor.matmul(prz, lhsT=ones_row, rhs=rz_row, start=True, stop=True)
        rzb_all = singles.tile([P, b_shard], FP32)
        nc.vector.tensor_copy(rzb_all, prz)
        for c in range(NCH):
            nc.vector.tensor_mul(
                attw_norm[:, c, :], attw_stage[:, c, :], rzb_all
            )
        nc.sync.dma_start(out=out_attw[:, :, :], in_=attw_norm)

    nc.compile()
    _build_cache[key] = nc
    return nc


def kernel(**inputs):
    from concourse.bass_utils import run_bass_kernel_spmd

    target = np.ascontiguousarray(np.asarray(inputs["target"], dtype=np.float32))
    sources = np.ascontiguousarray(np.asarray(inputs["sources"], dtype=np.float32))
    shared = {
        k: np.ascontiguousarray(np.asarray(inputs[k], dtype=np.float32))
        for k in ("W1_w", "W1_b", "W2_w", "W2_b", "V_w", "V_b")
    }

    nc = build_nc()
    in_maps = []
    for i in range(NCORES):
        sl = slice(i * B_SHARD, (i + 1) * B_SHARD)
        m = {"target": target[sl], "sources": sources[sl]}
        m.update(shared)
        in_maps.append(m)

    res = run_bass_kernel_spmd(nc, in_maps, core_ids=list(range(NCORES)))
    results = res.results

    ctx = np.concatenate([r["ctx"] for r in results], axis=0)
    attw = np.concatenate(
        [r["attw"].transpose(2, 1, 0).reshape(B_SHARD, N) for r in results], axis=0
    )
    return ctx.astype(np.float32), attw.astype(np.float32)[:, :, None]


# revision 24
# speedup vs baseline: 1.0383x; 1.0383x over previous
"""Bahdanau additive attention on 8 Trainium2 NeuronCores (data-parallel).

Reference computation (per batch item b):
    t[c]      = target[b] @ W1_w + W1_b                         # [C]
    s[n, c]   = sources[b] @ W2_w + W2_b                        # [N, C]
    score[n]  = (tanh(t + s)[n, :] @ V_w + V_b) / sqrt(C)       # [N]
    attw      = softmax(score)                                  # [N]
    ctx[h]    = sum_n attw[n] * sources[b, n, h]                # [H]

Strategy: shard the batch dim (1024) over 8 cores (128 items each),
replicate the small weights.  Per item everything stays on-chip:
  - sources loaded once via SWDGE cast-DMA (f32 -> bf16), natural [n, h]
    layout; also reused as the ctx-matmul rhs.
  - the contraction-over-h matmul needs sources transposed: done on the
    TensorEngine as a regular matmul against the identity (out =
    natT @ I), 128x128 blocks, evacuated PSUM -> SBUF bf16 by the DVE.
    (DMA-transpose would be cheaper on paper, but this Tile snapshot
    does not serialize the xbar transpose mode against plain DMA copies
    - a known hardware hang - so it is avoided entirely.)
  - s^T[c, n] accumulates in PSUM; ScalarE applies tanh with the
    per-partition bias t^T[c] + W1_b + W2_b fused in (and evacuates
    PSUM -> SBUF bf16).
  - score^T[n] via 16 small matmuls (lhsT = tanh tiles, rhs = V chunks).
  - exp on ScalarE with the 1/sqrt(C) scale and V_b bias folded in.
  - softmax skips the max-subtraction (|score| <= sum|V|/sqrt(C) < ~1,
    exp is safe and the result is mathematically identical).
  - ctx_u and Z = sum(e) via matmuls against the natural-layout sources;
    normalization on-chip with DVE reciprocal.
attw is staged on-chip in [p, c, b] layout (n = c*128 + p) and fixed up
with a host-side transpose after gathering.
"""

import math
import numpy as np

B, N, H, C = 1024, 512, 512, 512
NCORES = 8
P = 128
NCH, HCH, CCH = N // P, H // P, C // P
B_SHARD = B // NCORES

_build_cache = {}


def build_nc(b_shard=B_SHARD, enable_asserts=False):
    """Build (and bacc-compile) the per-core Bass graph."""
    key = (b_shard, enable_asserts)
    if key in _build_cache:
        return _build_cache[key]

    from contextlib import ExitStack

    import concourse.bass as bass
    import concourse.tile as tile
    from concourse import bacc, mybir

    FP32 = mybir.dt.float32
    BF16 = mybir.dt.bfloat16
    AF = mybir.ActivationFunctionType
    RSQRT_C = 1.0 / math.sqrt(C)

    nc = bacc.Bacc(
        "TRN2", target_bir_lowering=False, debug=False, enable_asserts=enable_asserts
    )

    tgt = nc.declare_dram_parameter("target", [b_shard, H], FP32, isOutput=False)
    src = nc.declare_dram_parameter("sources", [b_shard, N, H], FP32, isOutput=False)
    w1 = nc.declare_dram_parameter("W1_w", [H, C], FP32, isOutput=False)
    w1b = nc.declare_dram_parameter("W1_b", [C], FP32, isOutput=False)
    w2 = nc.declare_dram_parameter("W2_w", [H, C], FP32, isOutput=False)
    w2b = nc.declare_dram_parameter("W2_b", [C], FP32, isOutput=False)
    vw = nc.declare_dram_parameter("V_w", [C, 1], FP32, isOutput=False)
    vb = nc.declare_dram_parameter("V_b", [1], FP32, isOutput=False)
    out_ctx = nc.declare_dram_parameter("ctx", [b_shard, H], FP32, isOutput=True)
    out_attw = nc.declare_dram_parameter(
        "attw", [P, NCH, b_shard], FP32, isOutput=True
    )

    with tile.TileContext(nc) as tc, ExitStack() as ctx:
        singles = ctx.enter_context(tc.tile_pool(name="singles", bufs=1))
        natp = ctx.enter_context(tc.tile_pool(name="nat", bufs=3))
        srcTp = ctx.enter_context(tc.tile_pool(name="srcT", bufs=8))
        tanp = ctx.enter_context(tc.tile_pool(name="tanh", bufs=6))
        ep = ctx.enter_context(tc.tile_pool(name="e", bufs=3))
        ctxsbp = ctx.enter_context(tc.tile_pool(name="ctxsb", bufs=3))
        smallp = ctx.enter_context(tc.tile_pool(name="small", bufs=6))
        # PSUM pools: 3 + 2 + 1 + 1 = 7 banks of 8
        psp = ctx.enter_context(tc.tile_pool(name="ps_s", bufs=3, space="PSUM"))
        psTp = ctx.enter_context(tc.tile_pool(name="ps_T", bufs=2, space="PSUM"))
        scp = ctx.enter_context(tc.tile_pool(name="ps_sc", bufs=1, space="PSUM"))
        ctxp = ctx.enter_context(tc.tile_pool(name="ps_ctx", bufs=1, space="PSUM"))

        # ---- one-time per-core constants ----
        # W2 as bf16 lhsT blocks: w2_sb[p, k, i, c'] = W2[128k+p, 128i+c']
        w2_sb = singles.tile([P, HCH, CCH, P], BF16)
        nc.gpsimd.dma_start(
            out=w2_sb, in_=w2[:, :].rearrange("(k p) (i c) -> p k i c", p=P, c=P)
        )
        w1_sb = singles.tile([P, HCH, CCH, P], BF16)
        nc.gpsimd.dma_start(
            out=w1_sb, in_=w1[:, :].rearrange("(k p) (i c) -> p k i c", p=P, c=P)
        )
        # biases chunked [q, i] = b[128i + q]
        w1b_sb = singles.tile([P, CCH], FP32)
        nc.gpsimd.dma_start(out=w1b_sb, in_=w1b[:].rearrange("(i q) -> q i", q=P))
        w2b_sb = singles.tile([P, CCH], FP32)
        nc.gpsimd.dma_start(out=w2b_sb, in_=w2b[:].rearrange("(i q) -> q i", q=P))
        bsum = singles.tile([P, CCH], FP32)
        nc.vector.tensor_add(bsum, w1b_sb, w2b_sb)
        # V chunked [q, i] = V_w[128i + q]
        v_sb = singles.tile([P, CCH], BF16)
        nc.gpsimd.dma_start(out=v_sb, in_=vw[:, :].rearrange("(i q) o -> q (i o)", q=P))
        # V_b broadcast to all partitions, pre-scaled by 1/sqrt(C)
        vbs = singles.tile([P, 1], FP32)
        vb_ap = bass.AP(tensor=vb[:].tensor, offset=0, ap=[[0, P], [1, 1]])
        nc.gpsimd.dma_start(out=vbs, in_=vb_ap)
        nc.vector.tensor_scalar_mul(vbs, vbs, RSQRT_C)
        # constants for the Z-sum and the reciprocal broadcast
        ones_col = singles.tile([P, 1], BF16)
        nc.vector.memset(ones_col, 1.0)
        ones_row = singles.tile([1, P], FP32)
        nc.vector.memset(ones_row, 1.0)
        # per-item 1/Z collected here; attw is normalized once at the end
        rz_row = singles.tile([1, b_shard], FP32)
        # bf16 identity for TensorEngine transposes
        from concourse import masks

        idn = singles.tile([P, P], BF16)
        masks.make_identity(nc, idn[:])
        # attw staging: [p, c, b] = unnormalized exp(score), n = 128c+p
        attw_stage = singles.tile([P, NCH, b_shard], FP32)

        # ---- t^T = (target @ W1 + W1_b + W2_b)^T, [q, i, b] layout ----
        tgt_bf = singles.tile([b_shard, H], BF16)
        nc.gpsimd.dma_start(out=tgt_bf, in_=tgt[:, :])
        tgtT = singles.tile([P, HCH, b_shard], BF16)
        for k in range(HCH):
            ptT = psTp.tile([P, b_shard], FP32, tag="psT")
            nc.tensor.matmul(
                ptT,
                lhsT=tgt_bf[:, k * P : (k + 1) * P],
                rhs=idn[:b_shard, :b_shard],
                start=True,
                stop=True,
            )
            nc.vector.tensor_copy(tgtT[:, k, :], ptT)
        tT_sb = singles.tile([P, CCH, b_shard], FP32)
        for i in range(CCH):
            pt = psp.tile([P, b_shard], FP32, tag="ps")
            for k in range(HCH):
                nc.tensor.matmul(
                    pt,
                    lhsT=w1_sb[:, k, i, :],
                    rhs=tgtT[:, k, :],
                    start=(k == 0),
                    stop=(k == HCH - 1),
                )
            nc.scalar.activation(
                tT_sb[:, i, :], pt, AF.Identity, bias=bsum[:, i : i + 1]
            )

        # ---- per batch item ----
        for b in range(b_shard):
            # sources[b] in natural layout, bf16: nat[p, c, h] = src[b, 128c+p, h]
            nat = natp.tile([P, NCH, H], BF16)
            nc.gpsimd.dma_start(
                out=nat, in_=src[b].rearrange("(c p) h -> p c h", p=P)
            )
            # transposed: srcT_j[q, n] = src[b, n, 128j+q]
            # (regular matmul against identity: out = nat_block^T @ I)
            srcTs = []
            for j in range(HCH):
                psT = psTp.tile([P, N], FP32, tag="psT")
                for c in range(NCH):
                    nc.tensor.matmul(
                        psT[:, c * P : (c + 1) * P],
                        lhsT=nat[:, c, j * P : (j + 1) * P],
                        rhs=idn,
                        start=True,
                        stop=True,
                    )
                sT = srcTp.tile([P, N], BF16)
                nc.vector.tensor_copy(sT, psT)
                srcTs.append(sT)

            # score^T accumulator + Z + rz broadcast share one PSUM bank
            sc = scp.tile([P, NCH + 2], FP32)
            ths = []
            for i in range(CCH):
                ps = psp.tile([P, N], FP32, tag="ps")
                for j in range(HCH):
                    nc.tensor.matmul(
                        ps,
                        lhsT=w2_sb[:, j, i, :],
                        rhs=srcTs[j],
                        start=(j == 0),
                        stop=(j == HCH - 1),
                    )
                th = tanp.tile([P, N], BF16)
                nc.scalar.activation(th, ps, AF.Tanh, bias=tT_sb[:, i, b : b + 1])
                ths.append(th)
            # score column c must finish its accumulation group before the
            # next column starts (one pending group per PSUM bank).
            for c in range(NCH):
                for i in range(CCH):
                    nc.tensor.matmul(
                        sc[:, c : c + 1],
                        lhsT=ths[i][:, c * P : (c + 1) * P],
                        rhs=v_sb[:, i : i + 1],
                        start=(i == 0),
                        stop=(i == CCH - 1),
                    )

            # e = exp(score / sqrt(C) + V_b / sqrt(C)) written straight into
            # the (unnormalized) attw staging buffer; per-partition sums ride
            # along via accum_out, so Z needs only one 128->1 matmul.
            esum = ep.tile([P, 1], FP32)
            nc.scalar.activation(
                attw_stage[:, :, b],
                sc[:, 0:NCH],
                AF.Exp,
                bias=vbs[:, 0:1],
                scale=RSQRT_C,
                accum_out=esum,
            )
            e16 = ep.tile([P, NCH], BF16)
            nc.vector.tensor_copy(e16, attw_stage[:, :, b])
            esum16 = ep.tile([P, 1], BF16)
            nc.vector.tensor_copy(esum16, esum)

            # ctx_u[0, h] = sum_n e[n] src[b, n, h];  Z = sum_n e[n]
            pctx = ctxp.tile([1, H], FP32)
            for c in range(NCH):
                nc.tensor.matmul(
                    pctx,
                    lhsT=e16[:, c : c + 1],
                    rhs=nat[:, c, :],
                    start=(c == 0),
                    stop=(c == NCH - 1),
                )
            nc.tensor.matmul(
                sc[0:1, NCH : NCH + 1],
                lhsT=esum16,
                rhs=ones_col,
                start=True,
                stop=True,
            )
            rz = smallp.tile([1, 1], FP32)
            nc.vector.reciprocal(rz, sc[0:1, NCH : NCH + 1])
            nc.vector.tensor_copy(rz_row[0:1, b : b + 1], rz)
            ctx_sb = ctxsbp.tile([1, H], FP32)
            nc.vector.tensor_scalar_mul(ctx_sb, pctx, rz)
            nc.sync.dma_start(out=out_ctx[b : b + 1, :], in_=ctx_sb)

        # normalize attw once: stage[p, c, b] *= rz[b]
        prz = psTp.tile([P, b_shard], FP32, tag="psT")
        nc.tensor.matmul(prz, lhsT=ones_row, rhs=rz_row, start=True, stop=True)
        rzb_all = singles.tile([P, b_shard], FP32)
        nc.vector.tensor_copy(rzb_all, prz)
        for c in range(NCH):
            nc.vector.tensor_mul(
                attw_stage[:, c, :], attw_stage[:, c, :], rzb_all
            )
        nc.sync.dma_start(out=out_attw[:, :, :], in_=attw_stage)

    nc.compile()
    _build_cache[key] = nc
    return nc


def kernel(**inputs):
    from concourse.bass_utils import run_bass_kernel_spmd

    target = np.ascontiguousarray(np.asarray(inputs["target"], dtype=np.float32))
    sources = np.ascontiguousarray(np.asarray(inputs["sources"], dtype=np.float32))
    shared = {
        k: np.ascontiguousarray(np.asarray(inputs[k], dtype=np.float32))
        for k in ("W1_w", "W1_b", "W2_w", "W2_b", "V_w", "V_b")
    }

    nc = build_nc()
    in_maps = []
    for i in range(NCORES):
        sl = slice(i * B_SHARD, (i + 1) * B_SHARD)
        m = {"target": target[sl], "sources": sources[sl]}
        m.update(shared)
        in_maps.append(m)

    res = run_bass_kernel_spmd(nc, in_maps, core_ids=list(range(NCORES)))
    results = res.results

    ctx = np.concatenate([r["ctx"] for r in results], axis=0)
    attw = np.concatenate(
        [r["attw"].transpose(2, 1, 0).reshape(B_SHARD, N) for r in results], axis=0
    )
    return ctx.astype(np.float32), attw.astype(np.float32)[:, :, None]


# revision 26
# speedup vs baseline: 1.0753x; 1.0357x over previous
"""Bahdanau additive attention on 8 Trainium2 NeuronCores (data-parallel).

Reference computation (per batch item b):
    t[c]      = target[b] @ W1_w + W1_b                         # [C]
    s[n, c]   = sources[b] @ W2_w + W2_b                        # [N, C]
    score[n]  = (tanh(t + s)[n, :] @ V_w + V_b) / sqrt(C)       # [N]
    attw      = softmax(score)                                  # [N]
    ctx[h]    = sum_n attw[n] * sources[b, n, h]                # [H]

Strategy: shard the batch dim (1024) over 8 cores (128 items each),
replicate the small weights.  Per item everything stays on-chip:
  - sources loaded once via SWDGE cast-DMA (f32 -> bf16), natural [n, h]
    layout; also reused as the ctx-matmul rhs.
  - the contraction-over-h matmul needs sources transposed: done on the
    TensorEngine as a regular matmul against the identity (out =
    natT @ I), 128x128 blocks, evacuated PSUM -> SBUF bf16 by the DVE.
    (DMA-transpose would be cheaper on paper, but this Tile snapshot
    does not serialize the xbar transpose mode against plain DMA copies
    - a known hardware hang - so it is avoided entirely.)
  - s^T[c, n] accumulates in PSUM; ScalarE applies tanh with the
    per-partition bias t^T[c] + W1_b + W2_b fused in (and evacuates
    PSUM -> SBUF bf16).
  - score^T[n] via 16 small matmuls (lhsT = tanh tiles, rhs = V chunks).
  - exp on ScalarE with the 1/sqrt(C) scale and V_b bias folded in.
  - softmax skips the max-subtraction (|score| <= sum|V|/sqrt(C) < ~1,
    exp is safe and the result is mathematically identical).
  - ctx_u and Z = sum(e) via matmuls against the natural-layout sources;
    normalization on-chip with DVE reciprocal.
attw is staged on-chip in [p, c, b] layout (n = c*128 + p) and fixed up
with a host-side transpose after gathering.
"""

import math
import numpy as np

B, N, H, C = 1024, 512, 512, 512
NCORES = 8
P = 128
NCH, HCH, CCH = N // P, H // P, C // P
B_SHARD = B // NCORES

_build_cache = {}


def build_nc(b_shard=B_SHARD, enable_asserts=False):
    """Build (and bacc-compile) the per-core Bass graph."""
    key = (b_shard, enable_asserts)
    if key in _build_cache:
        return _build_cache[key]

    from contextlib import ExitStack

    import concourse.bass as bass
    import concourse.tile as tile
    from concourse import bacc, mybir

    FP32 = mybir.dt.float32
    BF16 = mybir.dt.bfloat16
    AF = mybir.ActivationFunctionType
    RSQRT_C = 1.0 / math.sqrt(C)

    nc = bacc.Bacc(
        "TRN2", target_bir_lowering=False, debug=False, enable_asserts=enable_asserts
    )

    tgt = nc.declare_dram_parameter("target", [b_shard, H], FP32, isOutput=False)
    src = nc.declare_dram_parameter("sources", [b_shard, N, H], FP32, isOutput=False)
    w1 = nc.declare_dram_parameter("W1_w", [H, C], FP32, isOutput=False)
    w1b = nc.declare_dram_parameter("W1_b", [C], FP32, isOutput=False)
    w2 = nc.declare_dram_parameter("W2_w", [H, C], FP32, isOutput=False)
    w2b = nc.declare_dram_parameter("W2_b", [C], FP32, isOutput=False)
    vw = nc.declare_dram_parameter("V_w", [C, 1], FP32, isOutput=False)
    vb = nc.declare_dram_parameter("V_b", [1], FP32, isOutput=False)
    out_ctx = nc.declare_dram_parameter("ctx", [b_shard, H], FP32, isOutput=True)
    out_attw = nc.declare_dram_parameter(
        "attw", [P, NCH, b_shard], FP32, isOutput=True
    )

    with tile.TileContext(nc) as tc, ExitStack() as ctx:
        singles = ctx.enter_context(tc.tile_pool(name="singles", bufs=1))
        natp = ctx.enter_context(tc.tile_pool(name="nat", bufs=3))
        srcTp = ctx.enter_context(tc.tile_pool(name="srcT", bufs=8))
        tanp = ctx.enter_context(tc.tile_pool(name="tanh", bufs=6))
        ep = ctx.enter_context(tc.tile_pool(name="e", bufs=3))
        ctxsbp = ctx.enter_context(tc.tile_pool(name="ctxsb", bufs=3))
        smallp = ctx.enter_context(tc.tile_pool(name="small", bufs=6))
        # PSUM pools: 3 + 3 + 1 + 1 = 8 banks of 8
        psp = ctx.enter_context(tc.tile_pool(name="ps_s", bufs=3, space="PSUM"))
        psTp = ctx.enter_context(tc.tile_pool(name="ps_T", bufs=3, space="PSUM"))
        scp = ctx.enter_context(tc.tile_pool(name="ps_sc", bufs=1, space="PSUM"))
        ctxp = ctx.enter_context(tc.tile_pool(name="ps_ctx", bufs=1, space="PSUM"))

        # ---- one-time per-core constants ----
        # W2 as bf16 lhsT blocks: w2_sb[p, k, i, c'] = W2[128k+p, 128i+c']
        w2_sb = singles.tile([P, HCH, CCH, P], BF16)
        nc.gpsimd.dma_start(
            out=w2_sb, in_=w2[:, :].rearrange("(k p) (i c) -> p k i c", p=P, c=P)
        )
        w1_sb = singles.tile([P, HCH, CCH, P], BF16)
        nc.gpsimd.dma_start(
            out=w1_sb, in_=w1[:, :].rearrange("(k p) (i c) -> p k i c", p=P, c=P)
        )
        # biases chunked [q, i] = b[128i + q]
        w1b_sb = singles.tile([P, CCH], FP32)
        nc.gpsimd.dma_start(out=w1b_sb, in_=w1b[:].rearrange("(i q) -> q i", q=P))
        w2b_sb = singles.tile([P, CCH], FP32)
        nc.gpsimd.dma_start(out=w2b_sb, in_=w2b[:].rearrange("(i q) -> q i", q=P))
        bsum = singles.tile([P, CCH], FP32)
        nc.vector.tensor_add(bsum, w1b_sb, w2b_sb)
        # V chunked [q, i] = V_w[128i + q]
        v_sb = singles.tile([P, CCH], BF16)
        nc.gpsimd.dma_start(out=v_sb, in_=vw[:, :].rearrange("(i q) o -> q (i o)", q=P))
        # V_b broadcast to all partitions, pre-scaled by 1/sqrt(C)
        vbs = singles.tile([P, 1], FP32)
        vb_ap = bass.AP(tensor=vb[:].tensor, offset=0, ap=[[0, P], [1, 1]])
        nc.gpsimd.dma_start(out=vbs, in_=vb_ap)
        nc.vector.tensor_scalar_mul(vbs, vbs, RSQRT_C)
        # constants for the Z-sum and the reciprocal broadcast
        ones_col = singles.tile([P, 1], BF16)
        nc.vector.memset(ones_col, 1.0)
        ones_row = singles.tile([1, P], FP32)
        nc.vector.memset(ones_row, 1.0)
        # per-item 1/Z collected here; attw is normalized once at the end
        rz_row = singles.tile([1, b_shard], FP32)
        # bf16 identity for TensorEngine transposes
        from concourse import masks

        idn = singles.tile([P, P], BF16)
        masks.make_identity(nc, idn[:])
        # attw staging: [p, c, b] = unnormalized exp(score), n = 128c+p
        attw_stage = singles.tile([P, NCH, b_shard], FP32)

        # ---- t^T = (target @ W1 + W1_b + W2_b)^T, [q, i, b] layout ----
        tgt_bf = singles.tile([b_shard, H], BF16)
        nc.gpsimd.dma_start(out=tgt_bf, in_=tgt[:, :])
        tgtT = singles.tile([P, HCH, b_shard], BF16)
        for k in range(HCH):
            ptT = psTp.tile([P, b_shard], FP32, tag="psT")
            nc.tensor.matmul(
                ptT,
                lhsT=tgt_bf[:, k * P : (k + 1) * P],
                rhs=idn[:b_shard, :b_shard],
                start=True,
                stop=True,
            )
            nc.vector.tensor_copy(tgtT[:, k, :], ptT)
        tT_sb = singles.tile([P, CCH, b_shard], FP32)
        for i in range(CCH):
            pt = psp.tile([P, b_shard], FP32, tag="ps")
            for k in range(HCH):
                nc.tensor.matmul(
                    pt,
                    lhsT=w1_sb[:, k, i, :],
                    rhs=tgtT[:, k, :],
                    start=(k == 0),
                    stop=(k == HCH - 1),
                )
            nc.scalar.activation(
                tT_sb[:, i, :], pt, AF.Identity, bias=bsum[:, i : i + 1]
            )

        # ---- per batch item ----
        for b in range(b_shard):
            # sources[b] in natural layout, bf16: nat[p, c, h] = src[b, 128c+p, h]
            nat = natp.tile([P, NCH, H], BF16)
            nc.gpsimd.dma_start(
                out=nat, in_=src[b].rearrange("(c p) h -> p c h", p=P)
            )
            # transposed: srcT_j[q, n] = src[b, n, 128j+q]
            # (regular matmul against identity: out = nat_block^T @ I)
            srcTs = []
            for j in range(HCH):
                psT = psTp.tile([P, N], FP32, tag="psT")
                for c in range(NCH):
                    nc.tensor.matmul(
                        psT[:, c * P : (c + 1) * P],
                        lhsT=nat[:, c, j * P : (j + 1) * P],
                        rhs=idn,
                        start=True,
                        stop=True,
                    )
                sT = srcTp.tile([P, N], BF16)
                # evacuation is the slow stage of the transpose sub-pipeline;
                # split it across DVE and ScalarE
                if j % 2 == 0:
                    nc.vector.tensor_copy(sT, psT)
                else:
                    nc.scalar.copy(sT, psT)
                srcTs.append(sT)

            # score^T accumulator + Z + rz broadcast share one PSUM bank
            sc = scp.tile([P, NCH + 2], FP32)
            ths = []
            for i in range(CCH):
                ps = psp.tile([P, N], FP32, tag="ps")
                for j in range(HCH):
                    nc.tensor.matmul(
                        ps,
                        lhsT=w2_sb[:, j, i, :],
                        rhs=srcTs[j],
                        start=(j == 0),
                        stop=(j == HCH - 1),
                    )
                th = tanp.tile([P, N], BF16)
                nc.scalar.activation(th, ps, AF.Tanh, bias=tT_sb[:, i, b : b + 1])
                ths.append(th)
            # score column c must finish its accumulation group before the
            # next column starts (one pending group per PSUM bank).
            for c in range(NCH):
                for i in range(CCH):
                    nc.tensor.matmul(
                        sc[:, c : c + 1],
                        lhsT=ths[i][:, c * P : (c + 1) * P],
                        rhs=v_sb[:, i : i + 1],
                        start=(i == 0),
                        stop=(i == CCH - 1),
                    )

            # e = exp(score / sqrt(C) + V_b / sqrt(C)) written straight into
            # the (unnormalized) attw staging buffer; per-partition sums ride
            # along via accum_out, so Z needs only one 128->1 matmul.
            esum = ep.tile([P, 1], FP32)
            nc.scalar.activation(
                attw_stage[:, :, b],
                sc[:, 0:NCH],
                AF.Exp,
                bias=vbs[:, 0:1],
                scale=RSQRT_C,
                accum_out=esum,
            )
            e16 = ep.tile([P, NCH], BF16)
            nc.vector.tensor_copy(e16, attw_stage[:, :, b])
            esum16 = ep.tile([P, 1], BF16)
            nc.vector.tensor_copy(esum16, esum)

            # ctx_u[0, h] = sum_n e[n] src[b, n, h];  Z = sum_n e[n]
            pctx = ctxp.tile([1, H], FP32)
            for c in range(NCH):
                nc.tensor.matmul(
                    pctx,
                    lhsT=e16[:, c : c + 1],
                    rhs=nat[:, c, :],
                    start=(c == 0),
                    stop=(c == NCH - 1),
                )
            nc.tensor.matmul(
                sc[0:1, NCH : NCH + 1],
                lhsT=esum16,
                rhs=ones_col,
                start=True,
                stop=True,
            )
            rz = smallp.tile([1, 1], FP32)
            nc.vector.reciprocal(rz, sc[0:1, NCH : NCH + 1])
            nc.vector.tensor_copy(rz_row[0:1, b : b + 1], rz)
            ctx_sb = ctxsbp.tile([1, H], FP32)
            nc.vector.tensor_scalar_mul(ctx_sb, pctx, rz)
            nc.sync.dma_start(out=out_ctx[b : b + 1, :], in_=ctx_sb)

        # normalize attw once: stage[p, c, b] *= rz[b]
        prz = psTp.tile([P, b_shard], FP32, tag="psT")
        nc.tensor.matmul(prz, lhsT=ones_row, rhs=rz_row, start=True, stop=True)
        rzb_all = singles.tile([P, b_shard], FP32)
        nc.vector.tensor_copy(rzb_all, prz)
        for c in range(NCH):
            nc.vector.tensor_mul(
                attw_stage[:, c, :], attw_stage[:, c, :], rzb_all
            )
        nc.sync.dma_start(out=out_attw[:, :, :], in_=attw_stage)

    nc.compile()
    _build_cache[key] = nc
    return nc


def kernel(**inputs):
    from concourse.bass_utils import run_bass_kernel_spmd

    target = np.ascontiguousarray(np.asarray(inputs["target"], dtype=np.float32))
    sources = np.ascontiguousarray(np.asarray(inputs["sources"], dtype=np.float32))
    shared = {
        k: np.ascontiguousarray(np.asarray(inputs[k], dtype=np.float32))
        for k in ("W1_w", "W1_b", "W2_w", "W2_b", "V_w", "V_b")
    }

    nc = build_nc()
    in_maps = []
    for i in range(NCORES):
        sl = slice(i * B_SHARD, (i + 1) * B_SHARD)
        m = {"target": target[sl], "sources": sources[sl]}
        m.update(shared)
        in_maps.append(m)

    res = run_bass_kernel_spmd(nc, in_maps, core_ids=list(range(NCORES)))
    results = res.results

    ctx = np.concatenate([r["ctx"] for r in results], axis=0)
    attw = np.concatenate(
        [r["attw"].transpose(2, 1, 0).reshape(B_SHARD, N) for r in results], axis=0
    )
    return ctx.astype(np.float32), attw.astype(np.float32)[:, :, None]


# revision 27
# speedup vs baseline: 1.0778x; 1.0024x over previous
"""Bahdanau additive attention on 8 Trainium2 NeuronCores (data-parallel).

Reference computation (per batch item b):
    t[c]      = target[b] @ W1_w + W1_b                         # [C]
    s[n, c]   = sources[b] @ W2_w + W2_b                        # [N, C]
    score[n]  = (tanh(t + s)[n, :] @ V_w + V_b) / sqrt(C)       # [N]
    attw      = softmax(score)                                  # [N]
    ctx[h]    = sum_n attw[n] * sources[b, n, h]                # [H]

Strategy: shard the batch dim (1024) over 8 cores (128 items each),
replicate the small weights.  Per item everything stays on-chip:
  - sources loaded once via SWDGE cast-DMA (f32 -> bf16), natural [n, h]
    layout; also reused as the ctx-matmul rhs.
  - the contraction-over-h matmul needs sources transposed: done on the
    TensorEngine as a regular matmul against the identity (out =
    natT @ I), 128x128 blocks, evacuated PSUM -> SBUF bf16 by the DVE.
    (DMA-transpose would be cheaper on paper, but this Tile snapshot
    does not serialize the xbar transpose mode against plain DMA copies
    - a known hardware hang - so it is avoided entirely.)
  - s^T[c, n] accumulates in PSUM; ScalarE applies tanh with the
    per-partition bias t^T[c] + W1_b + W2_b fused in (and evacuates
    PSUM -> SBUF bf16).
  - score^T[n] via 16 small matmuls (lhsT = tanh tiles, rhs = V chunks).
  - exp on ScalarE with the 1/sqrt(C) scale and V_b bias folded in.
  - softmax skips the max-subtraction (|score| <= sum|V|/sqrt(C) < ~1,
    exp is safe and the result is mathematically identical).
  - ctx_u and Z = sum(e) via matmuls against the natural-layout sources;
    normalization on-chip with DVE reciprocal.
attw is staged on-chip in [p, c, b] layout (n = c*128 + p) and fixed up
with a host-side transpose after gathering.
"""

import math
import numpy as np

B, N, H, C = 1024, 512, 512, 512
NCORES = 8
P = 128
NCH, HCH, CCH = N // P, H // P, C // P
B_SHARD = B // NCORES

_build_cache = {}


def build_nc(b_shard=B_SHARD, enable_asserts=False):
    """Build (and bacc-compile) the per-core Bass graph."""
    key = (b_shard, enable_asserts)
    if key in _build_cache:
        return _build_cache[key]

    from contextlib import ExitStack

    import concourse.bass as bass
    import concourse.tile as tile
    from concourse import bacc, mybir

    FP32 = mybir.dt.float32
    BF16 = mybir.dt.bfloat16
    AF = mybir.ActivationFunctionType
    RSQRT_C = 1.0 / math.sqrt(C)

    nc = bacc.Bacc(
        "TRN2", target_bir_lowering=False, debug=False, enable_asserts=enable_asserts
    )

    tgt = nc.declare_dram_parameter("target", [b_shard, H], FP32, isOutput=False)
    src = nc.declare_dram_parameter("sources", [b_shard, N, H], FP32, isOutput=False)
    w1 = nc.declare_dram_parameter("W1_w", [H, C], FP32, isOutput=False)
    w1b = nc.declare_dram_parameter("W1_b", [C], FP32, isOutput=False)
    w2 = nc.declare_dram_parameter("W2_w", [H, C], FP32, isOutput=False)
    w2b = nc.declare_dram_parameter("W2_b", [C], FP32, isOutput=False)
    vw = nc.declare_dram_parameter("V_w", [C, 1], FP32, isOutput=False)
    vb = nc.declare_dram_parameter("V_b", [1], FP32, isOutput=False)
    out_ctx = nc.declare_dram_parameter("ctx", [b_shard, H], FP32, isOutput=True)
    out_attw = nc.declare_dram_parameter(
        "attw", [P, NCH, b_shard], FP32, isOutput=True
    )

    with tile.TileContext(nc) as tc, ExitStack() as ctx:
        singles = ctx.enter_context(tc.tile_pool(name="singles", bufs=1))
        natp = ctx.enter_context(tc.tile_pool(name="nat", bufs=3))
        srcTp = ctx.enter_context(tc.tile_pool(name="srcT", bufs=8))
        tanp = ctx.enter_context(tc.tile_pool(name="tanh", bufs=6))
        ep = ctx.enter_context(tc.tile_pool(name="e", bufs=3))
        ctxsbp = ctx.enter_context(tc.tile_pool(name="ctxsb", bufs=3))
        smallp = ctx.enter_context(tc.tile_pool(name="small", bufs=6))
        # PSUM pools: 3 + 3 + 1 + 1 = 8 banks of 8
        psp = ctx.enter_context(tc.tile_pool(name="ps_s", bufs=3, space="PSUM"))
        psTp = ctx.enter_context(tc.tile_pool(name="ps_T", bufs=3, space="PSUM"))
        scp = ctx.enter_context(tc.tile_pool(name="ps_sc", bufs=1, space="PSUM"))
        ctxp = ctx.enter_context(tc.tile_pool(name="ps_ctx", bufs=1, space="PSUM"))

        # ---- one-time per-core constants ----
        # W2 as bf16 lhsT blocks: w2_sb[p, k, i, c'] = W2[128k+p, 128i+c']
        w2_sb = singles.tile([P, HCH, CCH, P], BF16)
        nc.gpsimd.dma_start(
            out=w2_sb, in_=w2[:, :].rearrange("(k p) (i c) -> p k i c", p=P, c=P)
        )
        w1_sb = singles.tile([P, HCH, CCH, P], BF16)
        nc.gpsimd.dma_start(
            out=w1_sb, in_=w1[:, :].rearrange("(k p) (i c) -> p k i c", p=P, c=P)
        )
        # biases chunked [q, i] = b[128i + q]
        w1b_sb = singles.tile([P, CCH], FP32)
        nc.gpsimd.dma_start(out=w1b_sb, in_=w1b[:].rearrange("(i q) -> q i", q=P))
        w2b_sb = singles.tile([P, CCH], FP32)
        nc.gpsimd.dma_start(out=w2b_sb, in_=w2b[:].rearrange("(i q) -> q i", q=P))
        bsum = singles.tile([P, CCH], FP32)
        nc.vector.tensor_add(bsum, w1b_sb, w2b_sb)
        # V chunked [q, i] = V_w[128i + q]
        v_sb = singles.tile([P, CCH], BF16)
        nc.gpsimd.dma_start(out=v_sb, in_=vw[:, :].rearrange("(i q) o -> q (i o)", q=P))
        # V_b broadcast to all partitions, pre-scaled by 1/sqrt(C)
        vbs = singles.tile([P, 1], FP32)
        vb_ap = bass.AP(tensor=vb[:].tensor, offset=0, ap=[[0, P], [1, 1]])
        nc.gpsimd.dma_start(out=vbs, in_=vb_ap)
        nc.vector.tensor_scalar_mul(vbs, vbs, RSQRT_C)
        # constants for the Z-sum and the reciprocal broadcast
        ones_col = singles.tile([P, 1], BF16)
        nc.vector.memset(ones_col, 1.0)
        ones_row = singles.tile([1, P], FP32)
        nc.vector.memset(ones_row, 1.0)
        # per-item 1/Z collected here; attw is normalized once at the end
        rz_row = singles.tile([1, b_shard], FP32)
        # bf16 identity for TensorEngine transposes
        from concourse import masks

        idn = singles.tile([P, P], BF16)
        masks.make_identity(nc, idn[:])
        # attw staging: [p, c, b] = unnormalized exp(score), n = 128c+p
        attw_stage = singles.tile([P, NCH, b_shard], FP32)

        # ---- t^T = (target @ W1 + W1_b + W2_b)^T, [q, i, b] layout ----
        tgt_bf = singles.tile([b_shard, H], BF16)
        nc.gpsimd.dma_start(out=tgt_bf, in_=tgt[:, :])
        tgtT = singles.tile([P, HCH, b_shard], BF16)
        for k in range(HCH):
            ptT = psTp.tile([P, b_shard], FP32, tag="psT")
            nc.tensor.matmul(
                ptT,
                lhsT=tgt_bf[:, k * P : (k + 1) * P],
                rhs=idn[:b_shard, :b_shard],
                start=True,
                stop=True,
            )
            nc.vector.tensor_copy(tgtT[:, k, :], ptT)
        tT_sb = singles.tile([P, CCH, b_shard], FP32)
        for i in range(CCH):
            pt = psp.tile([P, b_shard], FP32, tag="ps")
            for k in range(HCH):
                nc.tensor.matmul(
                    pt,
                    lhsT=w1_sb[:, k, i, :],
                    rhs=tgtT[:, k, :],
                    start=(k == 0),
                    stop=(k == HCH - 1),
                )
            nc.scalar.activation(
                tT_sb[:, i, :], pt, AF.Identity, bias=bsum[:, i : i + 1]
            )

        # ---- per batch item ----
        for b in range(b_shard):
            # sources[b] in natural layout, bf16: nat[p, c, h] = src[b, 128c+p, h]
            nat = natp.tile([P, NCH, H], BF16)
            nc.gpsimd.dma_start(
                out=nat, in_=src[b].rearrange("(c p) h -> p c h", p=P)
            )
            # transposed: srcT_j[q, n] = src[b, n, 128j+q]
            # (regular matmul against identity: out = nat_block^T @ I)
            srcTs = []
            for j in range(HCH):
                psT = psTp.tile([P, N], FP32, tag="psT")
                for c in range(NCH):
                    nc.tensor.matmul(
                        psT[:, c * P : (c + 1) * P],
                        lhsT=nat[:, c, j * P : (j + 1) * P],
                        rhs=idn,
                        start=True,
                        stop=True,
                    )
                sT = srcTp.tile([P, N], BF16)
                # evacuation is the slow stage of the transpose sub-pipeline;
                # split it 3:1 across DVE and ScalarE (ScalarE also runs the
                # tanh that releases the main-matmul banks - keep it light)
                if j % 4 == 3:
                    nc.scalar.copy(sT, psT)
                else:
                    nc.vector.tensor_copy(sT, psT)
                srcTs.append(sT)

            # score^T accumulator + Z + rz broadcast share one PSUM bank
            sc = scp.tile([P, NCH + 2], FP32)
            ths = []
            for i in range(CCH):
                ps = psp.tile([P, N], FP32, tag="ps")
                for j in range(HCH):
                    nc.tensor.matmul(
                        ps,
                        lhsT=w2_sb[:, j, i, :],
                        rhs=srcTs[j],
                        start=(j == 0),
                        stop=(j == HCH - 1),
                    )
                th = tanp.tile([P, N], BF16)
                nc.scalar.activation(th, ps, AF.Tanh, bias=tT_sb[:, i, b : b + 1])
                ths.append(th)
            # score column c must finish its accumulation group before the
            # next column starts (one pending group per PSUM bank).
            for c in range(NCH):
                for i in range(CCH):
                    nc.tensor.matmul(
                        sc[:, c : c + 1],
                        lhsT=ths[i][:, c * P : (c + 1) * P],
                        rhs=v_sb[:, i : i + 1],
                        start=(i == 0),
                        stop=(i == CCH - 1),
                    )

            # e = exp(score / sqrt(C) + V_b / sqrt(C)) written straight into
            # the (unnormalized) attw staging buffer; per-partition sums ride
            # along via accum_out, so Z needs only one 128->1 matmul.
            esum = ep.tile([P, 1], FP32)
            nc.scalar.activation(
                attw_stage[:, :, b],
                sc[:, 0:NCH],
                AF.Exp,
                bias=vbs[:, 0:1],
                scale=RSQRT_C,
                accum_out=esum,
            )
            e16 = ep.tile([P, NCH], BF16)
            nc.vector.tensor_copy(e16, attw_stage[:, :, b])
            esum16 = ep.tile([P, 1], BF16)
            nc.vector.tensor_copy(esum16, esum)

            # ctx_u[0, h] = sum_n e[n] src[b, n, h];  Z = sum_n e[n]
            pctx = ctxp.tile([1, H], FP32)
            for c in range(NCH):
                nc.tensor.matmul(
                    pctx,
                    lhsT=e16[:, c : c + 1],
                    rhs=nat[:, c, :],
                    start=(c == 0),
                    stop=(c == NCH - 1),
                )
            nc.tensor.matmul(
                sc[0:1, NCH : NCH + 1],
                lhsT=esum16,
                rhs=ones_col,
                start=True,
                stop=True,
            )
            rz = smallp.tile([1, 1], FP32)
            nc.vector.reciprocal(rz, sc[0:1, NCH : NCH + 1])
            nc.vector.tensor_copy(rz_row[0:1, b : b + 1], rz)
            ctx_sb = ctxsbp.tile([1, H], FP32)
            nc.vector.tensor_scalar_mul(ctx_sb, pctx, rz)
            nc.sync.dma_start(out=out_ctx[b : b + 1, :], in_=ctx_sb)

        # normalize attw once: stage[p, c, b] *= rz[b]
        prz = psTp.tile([P, b_shard], FP32, tag="psT")
        nc.tensor.matmul(prz, lhsT=ones_row, rhs=rz_row, start=True, stop=True)
        rzb_all = singles.tile([P, b_shard], FP32)
        nc.vector.tensor_copy(rzb_all, prz)
        for c in range(NCH):
            nc.vector.tensor_mul(
                attw_stage[:, c, :], attw_stage[:, c, :], rzb_all
            )
        nc.sync.dma_start(out=out_attw[:, :, :], in_=attw_stage)

    nc.compile()
    _build_cache[key] = nc
    return nc


def kernel(**inputs):
    from concourse.bass_utils import run_bass_kernel_spmd

    target = np.ascontiguousarray(np.asarray(inputs["target"], dtype=np.float32))
    sources = np.ascontiguousarray(np.asarray(inputs["sources"], dtype=np.float32))
    shared = {
        k: np.ascontiguousarray(np.asarray(inputs[k], dtype=np.float32))
        for k in ("W1_w", "W1_b", "W2_w", "W2_b", "V_w", "V_b")
    }

    nc = build_nc()
    in_maps = []
    for i in range(NCORES):
        sl = slice(i * B_SHARD, (i + 1) * B_SHARD)
        m = {"target": target[sl], "sources": sources[sl]}
        m.update(shared)
        in_maps.append(m)

    res = run_bass_kernel_spmd(nc, in_maps, core_ids=list(range(NCORES)))
    results = res.results

    ctx = np.concatenate([r["ctx"] for r in results], axis=0)
    attw = np.concatenate(
        [r["attw"].transpose(2, 1, 0).reshape(B_SHARD, N) for r in results], axis=0
    )
    return ctx.astype(np.float32), attw.astype(np.float32)[:, :, None]


# revision 28
# speedup vs baseline: 1.0845x; 1.0062x over previous
"""Bahdanau additive attention on 8 Trainium2 NeuronCores (data-parallel).

Reference computation (per batch item b):
    t[c]      = target[b] @ W1_w + W1_b                         # [C]
    s[n, c]   = sources[b] @ W2_w + W2_b                        # [N, C]
    score[n]  = (tanh(t + s)[n, :] @ V_w + V_b) / sqrt(C)       # [N]
    attw      = softmax(score)                                  # [N]
    ctx[h]    = sum_n attw[n] * sources[b, n, h]                # [H]

Strategy: shard the batch dim (1024) over 8 cores (128 items each),
replicate the small weights.  Per item everything stays on-chip:
  - sources loaded once via SWDGE cast-DMA (f32 -> bf16), natural [n, h]
    layout; also reused as the ctx-matmul rhs.
  - the contraction-over-h matmul needs sources transposed: done on the
    TensorEngine as a regular matmul against the identity (out =
    natT @ I), 128x128 blocks, evacuated PSUM -> SBUF bf16 by the DVE.
    (DMA-transpose would be cheaper on paper, but this Tile snapshot
    does not serialize the xbar transpose mode against plain DMA copies
    - a known hardware hang - so it is avoided entirely.)
  - s^T[c, n] accumulates in PSUM; ScalarE applies tanh with the
    per-partition bias t^T[c] + W1_b + W2_b fused in (and evacuates
    PSUM -> SBUF bf16).
  - score^T[n] via 16 small matmuls (lhsT = tanh tiles, rhs = V chunks).
  - exp on ScalarE with the 1/sqrt(C) scale and V_b bias folded in.
  - softmax skips the max-subtraction (|score| <= sum|V|/sqrt(C) < ~1,
    exp is safe and the result is mathematically identical).
  - ctx_u and Z = sum(e) via matmuls against the natural-layout sources;
    normalization on-chip with DVE reciprocal.
attw is staged on-chip in [p, c, b] layout (n = c*128 + p) and fixed up
with a host-side transpose after gathering.
"""

import math
import numpy as np

B, N, H, C = 1024, 512, 512, 512
NCORES = 8
P = 128
NCH, HCH, CCH = N // P, H // P, C // P
B_SHARD = B // NCORES

_build_cache = {}


def build_nc(b_shard=B_SHARD, enable_asserts=False):
    """Build (and bacc-compile) the per-core Bass graph."""
    key = (b_shard, enable_asserts)
    if key in _build_cache:
        return _build_cache[key]

    from contextlib import ExitStack

    import concourse.bass as bass
    import concourse.tile as tile
    from concourse import bacc, mybir

    FP32 = mybir.dt.float32
    BF16 = mybir.dt.bfloat16
    AF = mybir.ActivationFunctionType
    RSQRT_C = 1.0 / math.sqrt(C)

    nc = bacc.Bacc(
        "TRN2", target_bir_lowering=False, debug=False, enable_asserts=enable_asserts
    )

    tgt = nc.declare_dram_parameter("target", [b_shard, H], FP32, isOutput=False)
    src = nc.declare_dram_parameter("sources", [b_shard, N, H], FP32, isOutput=False)
    w1 = nc.declare_dram_parameter("W1_w", [H, C], FP32, isOutput=False)
    w1b = nc.declare_dram_parameter("W1_b", [C], FP32, isOutput=False)
    w2 = nc.declare_dram_parameter("W2_w", [H, C], FP32, isOutput=False)
    w2b = nc.declare_dram_parameter("W2_b", [C], FP32, isOutput=False)
    vw = nc.declare_dram_parameter("V_w", [C, 1], FP32, isOutput=False)
    vb = nc.declare_dram_parameter("V_b", [1], FP32, isOutput=False)
    out_ctx = nc.declare_dram_parameter("ctx", [b_shard, H], FP32, isOutput=True)
    out_attw = nc.declare_dram_parameter(
        "attw", [P, NCH, b_shard], FP32, isOutput=True
    )

    with tile.TileContext(nc) as tc, ExitStack() as ctx:
        singles = ctx.enter_context(tc.tile_pool(name="singles", bufs=1))
        natp = ctx.enter_context(tc.tile_pool(name="nat", bufs=4))
        srcTp = ctx.enter_context(tc.tile_pool(name="srcT", bufs=12))
        tanp = ctx.enter_context(tc.tile_pool(name="tanh", bufs=8))
        ep = ctx.enter_context(tc.tile_pool(name="e", bufs=4))
        ctxsbp = ctx.enter_context(tc.tile_pool(name="ctxsb", bufs=3))
        smallp = ctx.enter_context(tc.tile_pool(name="small", bufs=6))
        # PSUM pools: 3 + 3 + 1 + 1 = 8 banks of 8
        psp = ctx.enter_context(tc.tile_pool(name="ps_s", bufs=3, space="PSUM"))
        psTp = ctx.enter_context(tc.tile_pool(name="ps_T", bufs=3, space="PSUM"))
        scp = ctx.enter_context(tc.tile_pool(name="ps_sc", bufs=1, space="PSUM"))
        ctxp = ctx.enter_context(tc.tile_pool(name="ps_ctx", bufs=1, space="PSUM"))

        # ---- one-time per-core constants ----
        # W2 as bf16 lhsT blocks: w2_sb[p, k, i, c'] = W2[128k+p, 128i+c']
        w2_sb = singles.tile([P, HCH, CCH, P], BF16)
        nc.gpsimd.dma_start(
            out=w2_sb, in_=w2[:, :].rearrange("(k p) (i c) -> p k i c", p=P, c=P)
        )
        w1_sb = singles.tile([P, HCH, CCH, P], BF16)
        nc.gpsimd.dma_start(
            out=w1_sb, in_=w1[:, :].rearrange("(k p) (i c) -> p k i c", p=P, c=P)
        )
        # biases chunked [q, i] = b[128i + q]
        w1b_sb = singles.tile([P, CCH], FP32)
        nc.gpsimd.dma_start(out=w1b_sb, in_=w1b[:].rearrange("(i q) -> q i", q=P))
        w2b_sb = singles.tile([P, CCH], FP32)
        nc.gpsimd.dma_start(out=w2b_sb, in_=w2b[:].rearrange("(i q) -> q i", q=P))
        bsum = singles.tile([P, CCH], FP32)
        nc.vector.tensor_add(bsum, w1b_sb, w2b_sb)
        # V chunked [q, i] = V_w[128i + q]
        v_sb = singles.tile([P, CCH], BF16)
        nc.gpsimd.dma_start(out=v_sb, in_=vw[:, :].rearrange("(i q) o -> q (i o)", q=P))
        # V_b broadcast to all partitions, pre-scaled by 1/sqrt(C)
        vbs = singles.tile([P, 1], FP32)
        vb_ap = bass.AP(tensor=vb[:].tensor, offset=0, ap=[[0, P], [1, 1]])
        nc.gpsimd.dma_start(out=vbs, in_=vb_ap)
        nc.vector.tensor_scalar_mul(vbs, vbs, RSQRT_C)
        # constants for the Z-sum and the reciprocal broadcast
        ones_col = singles.tile([P, 1], BF16)
        nc.vector.memset(ones_col, 1.0)
        ones_row = singles.tile([1, P], FP32)
        nc.vector.memset(ones_row, 1.0)
        # per-item 1/Z collected here; attw is normalized once at the end
        rz_row = singles.tile([1, b_shard], FP32)
        # bf16 identity for TensorEngine transposes
        from concourse import masks

        idn = singles.tile([P, P], BF16)
        masks.make_identity(nc, idn[:])
        # attw staging: [p, c, b] = unnormalized exp(score), n = 128c+p
        attw_stage = singles.tile([P, NCH, b_shard], FP32)

        # ---- t^T = (target @ W1 + W1_b + W2_b)^T, [q, i, b] layout ----
        tgt_bf = singles.tile([b_shard, H], BF16)
        nc.gpsimd.dma_start(out=tgt_bf, in_=tgt[:, :])
        tgtT = singles.tile([P, HCH, b_shard], BF16)
        for k in range(HCH):
            ptT = psTp.tile([P, b_shard], FP32, tag="psT")
            nc.tensor.matmul(
                ptT,
                lhsT=tgt_bf[:, k * P : (k + 1) * P],
                rhs=idn[:b_shard, :b_shard],
                start=True,
                stop=True,
            )
            nc.vector.tensor_copy(tgtT[:, k, :], ptT)
        tT_sb = singles.tile([P, CCH, b_shard], FP32)
        for i in range(CCH):
            pt = psp.tile([P, b_shard], FP32, tag="ps")
            for k in range(HCH):
                nc.tensor.matmul(
                    pt,
                    lhsT=w1_sb[:, k, i, :],
                    rhs=tgtT[:, k, :],
                    start=(k == 0),
                    stop=(k == HCH - 1),
                )
            nc.scalar.activation(
                tT_sb[:, i, :], pt, AF.Identity, bias=bsum[:, i : i + 1]
            )

        # ---- per batch item ----
        for b in range(b_shard):
            # sources[b] in natural layout, bf16: nat[p, c, h] = src[b, 128c+p, h]
            nat = natp.tile([P, NCH, H], BF16)
            nc.gpsimd.dma_start(
                out=nat, in_=src[b].rearrange("(c p) h -> p c h", p=P)
            )
            # transposed: srcT_j[q, n] = src[b, n, 128j+q]
            # (regular matmul against identity: out = nat_block^T @ I)
            srcTs = []
            for j in range(HCH):
                psT = psTp.tile([P, N], FP32, tag="psT")
                for c in range(NCH):
                    nc.tensor.matmul(
                        psT[:, c * P : (c + 1) * P],
                        lhsT=nat[:, c, j * P : (j + 1) * P],
                        rhs=idn,
                        start=True,
                        stop=True,
                    )
                sT = srcTp.tile([P, N], BF16)
                # evacuation is the slow stage of the transpose sub-pipeline;
                # split it 3:1 across DVE and ScalarE (ScalarE also runs the
                # tanh that releases the main-matmul banks - keep it light)
                if j % 4 == 3:
                    nc.scalar.copy(sT, psT)
                else:
                    nc.vector.tensor_copy(sT, psT)
                srcTs.append(sT)

            # score^T accumulator + Z + rz broadcast share one PSUM bank
            sc = scp.tile([P, NCH + 2], FP32)
            ths = []
            for i in range(CCH):
                ps = psp.tile([P, N], FP32, tag="ps")
                for j in range(HCH):
                    nc.tensor.matmul(
                        ps,
                        lhsT=w2_sb[:, j, i, :],
                        rhs=srcTs[j],
                        start=(j == 0),
                        stop=(j == HCH - 1),
                    )
                th = tanp.tile([P, N], BF16)
                nc.scalar.activation(th, ps, AF.Tanh, bias=tT_sb[:, i, b : b + 1])
                ths.append(th)
            # score column c must finish its accumulation group before the
            # next column starts (one pending group per PSUM bank).
            for c in range(NCH):
                for i in range(CCH):
                    nc.tensor.matmul(
                        sc[:, c : c + 1],
                        lhsT=ths[i][:, c * P : (c + 1) * P],
                        rhs=v_sb[:, i : i + 1],
                        start=(i == 0),
                        stop=(i == CCH - 1),
                    )

            # e = exp(score / sqrt(C) + V_b / sqrt(C)) written straight into
            # the (unnormalized) attw staging buffer; per-partition sums ride
            # along via accum_out, so Z needs only one 128->1 matmul.
            esum = ep.tile([P, 1], FP32)
            nc.scalar.activation(
                attw_stage[:, :, b],
                sc[:, 0:NCH],
                AF.Exp,
                bias=vbs[:, 0:1],
                scale=RSQRT_C,
                accum_out=esum,
            )
            e16 = ep.tile([P, NCH], BF16)
            nc.vector.tensor_copy(e16, attw_stage[:, :, b])
            esum16 = ep.tile([P, 1], BF16)
            nc.vector.tensor_copy(esum16, esum)

            # ctx_u[0, h] = sum_n e[n] src[b, n, h];  Z = sum_n e[n]
            pctx = ctxp.tile([1, H], FP32)
            for c in range(NCH):
                nc.tensor.matmul(
                    pctx,
                    lhsT=e16[:, c : c + 1],
                    rhs=nat[:, c, :],
                    start=(c == 0),
                    stop=(c == NCH - 1),
                )
            nc.tensor.matmul(
                sc[0:1, NCH : NCH + 1],
                lhsT=esum16,
                rhs=ones_col,
                start=True,
                stop=True,
            )
            rz = smallp.tile([1, 1], FP32)
            nc.vector.reciprocal(rz, sc[0:1, NCH : NCH + 1])
            nc.vector.tensor_copy(rz_row[0:1, b : b + 1], rz)
            ctx_sb = ctxsbp.tile([1, H], FP32)
            nc.vector.tensor_scalar_mul(ctx_sb, pctx, rz)
            nc.sync.dma_start(out=out_ctx[b : b + 1, :], in_=ctx_sb)

        # normalize attw once: stage[p, c, b] *= rz[b]
        prz = psTp.tile([P, b_shard], FP32, tag="psT")
        nc.tensor.matmul(prz, lhsT=ones_row, rhs=rz_row, start=True, stop=True)
        rzb_all = singles.tile([P, b_shard], FP32)
        nc.vector.tensor_copy(rzb_all, prz)
        for c in range(NCH):
            nc.vector.tensor_mul(
                attw_stage[:, c, :], attw_stage[:, c, :], rzb_all
            )
        nc.sync.dma_start(out=out_attw[:, :, :], in_=attw_stage)

    nc.compile()
    _build_cache[key] = nc
    return nc


def kernel(**inputs):
    from concourse.bass_utils import run_bass_kernel_spmd

    target = np.ascontiguousarray(np.asarray(inputs["target"], dtype=np.float32))
    sources = np.ascontiguousarray(np.asarray(inputs["sources"], dtype=np.float32))
    shared = {
        k: np.ascontiguousarray(np.asarray(inputs[k], dtype=np.float32))
        for k in ("W1_w", "W1_b", "W2_w", "W2_b", "V_w", "V_b")
    }

    nc = build_nc()
    in_maps = []
    for i in range(NCORES):
        sl = slice(i * B_SHARD, (i + 1) * B_SHARD)
        m = {"target": target[sl], "sources": sources[sl]}
        m.update(shared)
        in_maps.append(m)

    res = run_bass_kernel_spmd(nc, in_maps, core_ids=list(range(NCORES)))
    results = res.results

    ctx = np.concatenate([r["ctx"] for r in results], axis=0)
    attw = np.concatenate(
        [r["attw"].transpose(2, 1, 0).reshape(B_SHARD, N) for r in results], axis=0
    )
    return ctx.astype(np.float32), attw.astype(np.float32)[:, :, None]
